# revision 51
# baseline (speedup 1.0000x reference)
"""Trainium2 Bass kernel for NT-Xent contrastive loss (BATCH=4096, DIM=512, TEMP=0.5).

Strategy — exploit the symmetry of the similarity matrix + fp8 DoubleRow:
  - Host: L2-normalize rows of E = concat(emb_i, emb_j) in f32, compute the
    positive-pair dots and the (quantized) diagonal terms exactly, then cast
    z*16 to TRN fp8-e4m3 for the big matmul.
  - The 8192x8192 exp(sim/T) row-sum is split by symmetry: the 64x64 grid of
    128x128 tiles is covered by giving each block-row r the cyclic strip of
    tiles (r, r+c mod 64) for c = 0..32.  Tiles c = 1..31 contribute their
    row-sums to block r's denominators AND their column-sums (via symmetry
    s_ij = s_ji) to the denominators of rows in block r+c.  Tiles c = 0 and
    c = 32 are row-sum only (c = 32 appears in both orderings' strips).
    Every ordered pair (i, j) is covered exactly once; the self term
    exp(s_ii/T) is subtracted on host.
  - Core k owns block-rows 8k..8k+7; its rhs is the 5120-column cyclic window
    of z^T starting at column 1024k, so every core runs the IDENTICAL program
    on its own data (SPMD, no collectives).  The matmul weights are read
    straight out of the rhs tile (cols [0,1024) hold the core's own rows).
  - Device: fp8 DoubleRow matmuls (contraction 512 = 2 pairs of 128x2) into
    [128, 2048] PSUM chunks -> ACT exp (scale 2/256) into a bf16 strip
    buffer.  Row sums via DVE pairwise bf16 tree folds (2X mode) + small f32
    reduce.  Column sums as a short PE tail: 4 concurrent M=32 all-ones
    matmuls on distinct PE column groups (col tiling), strips accumulated in
    shared PSUM over three column phases, drained by [128,512] DVE copies
    and partition-batched DMAs.
  - Host: den = rowsum + colsum - diag; loss = mean(log(den) - pos/TEMP).
"""

import ml_dtypes
import numpy as np

BATCH = 4096
DIM = 512
TEMP = 0.5
B2 = 2 * BATCH            # 8192 rows of the similarity matrix
NCORES = 8
NBLK = B2 // 128          # 64 block-rows
SPB = NBLK // NCORES      # 8 strips (block-rows) per core
NT = 33                   # tiles per strip (c = 0..32)
SW = NT * 128             # 4224 strip width (stream columns per strip)
TOTAL = SPB * SW          # 33792 stream columns per core
LCOLS = 128 * (SPB - 1) + SW   # 5120 local rhs columns per core
SCALE = 16.0              # fp8 pre-scale on z
ACT_SCALE = (1.0 / TEMP) / (SCALE * SCALE)   # exp(s_hat * ACT_SCALE)
CS_LO = 128               # colsum window (local cols): [128j+128, 128j+4096)
CS_HI = 128 * (SPB - 1) + 4096   # 4992
CSW = CS_HI - CS_LO       # 4864
CPH = [(4224, 4992), (128, 2176), (2176, 4224)]   # colsum column phases
ROUNDS = [(0, 512), (512, 1024), (1024, 2048), (2048, 3072),
          (3072, 4224), (4224, LCOLS)]

_CACHE = {}


def _build():
    import concourse.bacc as bacc
    import concourse.mybir as mybir
    import concourse.tile as tile

    f32 = mybir.dt.float32
    bf16 = mybir.dt.bfloat16
    fp8 = mybir.dt.float8e4
    AF = mybir.ActivationFunctionType
    ALU = mybir.AluOpType
    X = mybir.AxisListType.X
    DR = mybir.MatmulPerfMode.DoubleRow

    nc = bacc.Bacc("TRN2", target_bir_lowering=False, debug=False,
                   num_devices=NCORES)

    xq_d = nc.dram_tensor("xq", [128, 4 * LCOLS], fp8, kind="ExternalInput").ap()
    rowout_d = nc.dram_tensor("rowout", [128, SPB], f32,
                              kind="ExternalOutput").ap()
    colout_d = nc.dram_tensor("colout", [1, CSW], f32,
                              kind="ExternalOutput").ap()

    with tile.TileContext(nc) as tc:
        with (
            tc.tile_pool(name="persist", bufs=1) as P,
            tc.tile_pool(name="scratch", bufs=2) as S,
            tc.tile_pool(name="psum", bufs=2, space="PSUM") as PS,
        ):
            ones = P.tile([128, 128], bf16, name="ones")
            xq = P.tile([128, 4 * LCOLS], fp8, name="xq")
            exps = P.tile([128, TOTAL], bf16, name="exps")
            rowacc = P.tile([128, SPB], f32, name="rowacc")
            colsb = P.tile([128, 512 * len(CPH)], f32, name="colsb")
            fa7 = P.tile([128, 1056], bf16, name="fa7")

            nc.vector.memset(ones[:], 1.0)
            # HBM loads, first-needed first, round-robined over three DMA
            # queues (the scalar DGE is idle until the first activation).
            qs = [nc.sync, nc.gpsimd, nc.scalar]
            qi = 0
            for lo, hi in ROUNDS:
                for s in range(4):
                    sl = slice(LCOLS * s + lo, LCOLS * s + hi)
                    qs[qi % 3].dma_start(xq[:, sl], xq_d[:, sl])
                    qi += 1

            # warm the PE HAM clock gate while the first loads are in flight
            wps = PS.tile([128, 128], f32, tag="mm", name="wps")
            for _ in range(24):
                nc.tensor.matmul(wps[:], ones[:], ones[:], start=True, stop=True)

            xq3 = xq[:].rearrange("p (s c) -> p s c", s=4)

            def fold(j, dst_ap, q):
                """DVE pairwise bf16 tree folds for strip j row sums.
                q = (q0+q2, q1+q3) quarters order so half the work can be
                emitted a chunk early for the final strip."""
                a = exps[:, SW * j:SW * (j + 1)]
                with nc.allow_low_precision(
                        reason="pairwise bf16 folds; final add is f32"):
                    f2 = S.tile([128, 1056], bf16, tag="f2", name="f2")
                    if q is None:
                        f1 = S.tile([128, 2112], bf16, tag="f1", name="f1")
                        nc.vector.tensor_tensor(
                            f1[:], a[:, 0:2112], a[:, 2112:4224], ALU.add)
                        nc.vector.tensor_tensor(
                            f2[:], f1[:, 0:1056], f1[:, 1056:2112], ALU.add)
                    else:
                        fb = S.tile([128, 1056], bf16, tag="f1", name="fb")
                        nc.vector.tensor_tensor(
                            fb[:], a[:, 1056:2112], a[:, 3168:4224], ALU.add)
                        nc.vector.tensor_tensor(f2[:], q[:], fb[:], ALU.add)
                    f3 = S.tile([128, 528], bf16, tag="f3", name="f3")
                    nc.vector.tensor_tensor(
                        f3[:], f2[:, 0:528], f2[:, 528:1056], ALU.add)
                nc.vector.tensor_reduce(dst_ap, f3[:], X, ALU.add)

            bounds = ([0, 512, 1024, 1536, 2048, 3072] +
                      list(range(4096, 32768 + 1, 2048)) + [33280, TOTAL])
            cuts = sorted(set(
                [512 * m for m in range(TOTAL // 512 + 1)] +
                [SW * j for j in range(SPB + 1)]))
            pieces = list(zip(cuts, cuts[1:]))
            reduced = set()
            fa_done = False
            for c0, c1 in zip(bounds, bounds[1:]):
                cp = [pc for pc in pieces if pc[0] >= c0 and pc[1] <= c1]
                ps = PS.tile([128, c1 - c0], f32, tag="mm", name="ps")
                emit = []
                for j in sorted(set(a // SW for a, b in cp)):
                    for kk in range(2):
                        for (a, b) in cp:
                            if a // SW == j:
                                emit.append((j, kk, a, b))
                regions = {}
                for idx, (j, kk, a, b) in enumerate(emit):
                    regions.setdefault(a // 512, []).append(idx)
                starts = {v[0] for v in regions.values()}
                stops = {v[-1] for v in regions.values()}
                for idx, (j, kk, a, b) in enumerate(emit):
                    lc = a - 4096 * j
                    nc.tensor.matmul(
                        ps[:, a - c0:b - c0],
                        xq3[:, 2 * kk:2 * kk + 2, 128 * j:128 * (j + 1)],
                        xq3[:, 2 * kk:2 * kk + 2, lc:lc + (b - a)],
                        start=(idx in starts), stop=(idx in stops),
                        perf_mode=DR)
                nc.scalar.activation(exps[:, c0:c1], ps[:, 0:c1 - c0],
                                     AF.Exp, scale=ACT_SCALE)
                for j in range(SPB - 1):
                    if j not in reduced and SW * (j + 1) <= c1:
                        fold(j, rowacc[:, j:j + 1], None)
                        reduced.add(j)
                if not fa_done and SW * (SPB - 1) + 3168 <= c1:
                    # front-load half of the last strip's fold: q0 + q2
                    b7 = SW * (SPB - 1)
                    with nc.allow_low_precision(reason="bf16 fold half"):
                        nc.vector.tensor_tensor(
                            fa7[:], exps[:, b7:b7 + 1056],
                            exps[:, b7 + 2112:b7 + 3168], ALU.add)
                    fa_done = True

            # column-sum tail: 4 concurrent M=32 all-ones matmuls on distinct
            # PE column groups, strips accumulated in shared PSUM per phase.
            # The last strip's remaining fold is slotted between phase 1 and
            # phase 2 so neither the copies nor the row sums gate the end.
            for p, (pa, pb) in enumerate(CPH):
                if p == 2:
                    fold(SPB - 1, rowacc[:, SPB - 1:SPB], fa7)
                    nc.gpsimd.dma_start(rowout_d[:], rowacc[:])
                cps = PS.tile([128, 512], f32, tag="mm", name="cps")
                emit = []
                for j in range(SPB):
                    for g in range(4):
                        ga, gb = pa + 512 * g, min(pb, pa + 512 * (g + 1))
                        wa = max(ga, 128 * j + 128)
                        wb = min(gb, 128 * j + 4096)
                        if wa < wb:
                            emit.append((g, j, wa, wb))
                groups = {}
                for idx, (g, j, a, b) in enumerate(emit):
                    groups.setdefault(g, []).append(idx)
                starts = {v[0] for v in groups.values()}
                stops = {v[-1] for v in groups.values()}
                for idx, (g, j, a, b) in enumerate(emit):
                    ga = pa + 512 * g
                    nc.tensor.matmul(
                        cps[32 * g:32 * (g + 1), a - ga:b - ga],
                        ones[:, 0:32], exps[:, 4096 * j + a:4096 * j + b],
                        start=(idx in starts), stop=(idx in stops),
                        tile_position=(0, 32 * g))
                csl = 512 * p
                # scalar engine is idle once the last exp chunk is done;
                # draining psum there keeps DVE free for the row-sum folds
                nc.scalar.copy(colsb[:, csl:csl + 512], cps[:])
                w = pb - pa
                eng = nc.sync if p % 2 == 0 else nc.gpsimd
                if w == 2048:
                    src = colsb[:].rearrange("(a b) f -> a b f", b=32)[
                        :, 0:1, csl:csl + 512]
                    eng.dma_start(colout_d[0:1, pa - CS_LO:pb - CS_LO], src)
                else:
                    for g in range(4):
                        ga, gb = pa + 512 * g, min(pb, pa + 512 * (g + 1))
                        if ga >= gb:
                            continue
                        eng.dma_start(
                            colout_d[0:1, ga - CS_LO:gb - CS_LO],
                            colsb[32 * g:32 * g + 1, csl:csl + (gb - ga)])

    nc.compile()
    return nc


def _get_nc():
    if "nc" not in _CACHE:
        _CACHE["nc"] = _build()
    return _CACHE["nc"]


def _prep(emb_i, emb_j):
    fp8 = ml_dtypes.float8_e4m3
    E = np.concatenate([np.asarray(emb_i, dtype=np.float32),
                        np.asarray(emb_j, dtype=np.float32)], axis=0)
    nrm = np.sqrt((E * E).sum(axis=1, keepdims=True))
    Z = E / np.maximum(nrm, 1e-12)                       # [8192, 512] f32
    pos = (Z[:BATCH] * Z[BATCH:]).sum(axis=1)
    posf = np.concatenate([pos, pos]) / TEMP             # [8192]
    Zq = (Z * SCALE).astype(fp8)                         # [8192, 512] fp8
    Zqf = Zq.astype(np.float32) / SCALE
    diag = np.exp((Zqf * Zqf).sum(axis=1) / TEMP)        # [8192]
    ZqT = np.ascontiguousarray(Zq.T)                     # [512, 8192]
    maps = []
    for k in range(NCORES):
        cols = (1024 * k + np.arange(LCOLS)) % B2
        Xc = ZqT[:, cols]                                # [512, 5120]
        xq = np.ascontiguousarray(
            Xc.reshape(4, 128, LCOLS).transpose(1, 0, 2).reshape(128, -1))
        maps.append({"xq": xq})
    return maps, posf, diag


def _run(emb_i, emb_j, trace=False):
    from concourse.bass_utils import run_bass_kernel_spmd
    nc = _get_nc()
    maps, posf, diag = _prep(emb_i, emb_j)
    res = run_bass_kernel_spmd(nc, maps, list(range(NCORES)), trace=trace)
    den = np.zeros(B2, dtype=np.float64)
    for k in range(NCORES):
        rowout = np.asarray(res.results[k]["rowout"], dtype=np.float64)
        colout = np.asarray(res.results[k]["colout"], dtype=np.float64)[0]
        rows = 1024 * k + np.arange(1024)
        den[rows] += rowout.T.reshape(-1)                # [p, j] -> row 128j+p
        g = (1024 * k + CS_LO + np.arange(CSW)) % B2
        den[g] += colout
    den = den - diag
    loss = np.float32(np.mean(np.log(den) - posf))
    return loss, res


def kernel(emb_i, emb_j):
    return _run(emb_i, emb_j, trace=False)[0]


# revision 53
# speedup vs baseline: 1.0548x; 1.0548x over previous
"""Trainium2 Bass kernel for NT-Xent contrastive loss (BATCH=4096, DIM=512, TEMP=0.5).

Strategy — exploit the symmetry of the similarity matrix + fp8 DoubleRow:
  - Host: L2-normalize rows of E = concat(emb_i, emb_j) in f32, compute the
    positive-pair dots and the (quantized) diagonal terms exactly, then cast
    z*16 to TRN fp8-e4m3 for the big matmul.
  - The 8192x8192 exp(sim/T) row-sum is split by symmetry: the 64x64 grid of
    128x128 tiles is covered by giving each block-row r the cyclic strip of
    tiles (r, r+c mod 64) for c = 0..32.  Tiles c = 1..31 contribute their
    row-sums to block r's denominators AND their column-sums (via symmetry
    s_ij = s_ji) to the denominators of rows in block r+c.  Tiles c = 0 and
    c = 32 are row-sum only (c = 32 appears in both orderings' strips).
    Every ordered pair (i, j) is covered exactly once; the self term
    exp(s_ii/T) is subtracted on host.
  - Core k owns block-rows 8k..8k+7; its rhs is the 5120-column cyclic window
    of z^T starting at column 1024k, so every core runs the IDENTICAL program
    on its own data (SPMD, no collectives).  The matmul weights are read
    straight out of the rhs tile (cols [0,1024) hold the core's own rows).
  - Device: fp8 DoubleRow matmuls (contraction 512 = 2 pairs of 128x2) into
    [128, 2048] PSUM chunks -> ACT exp (scale 2/256) into a bf16 strip
    buffer.  Row sums via DVE pairwise bf16 tree folds (2X mode) + small f32
    reduce.  Column sums as a short PE tail: 4 concurrent M=32 all-ones
    matmuls on distinct PE column groups (col tiling), strips accumulated in
    shared PSUM over three column phases, drained by [128,512] DVE copies
    and partition-batched DMAs.
  - Host: den = rowsum + colsum - diag; loss = mean(log(den) - pos/TEMP).
"""

import ml_dtypes
import numpy as np

BATCH = 4096
DIM = 512
TEMP = 0.5
B2 = 2 * BATCH            # 8192 rows of the similarity matrix
NCORES = 8
NBLK = B2 // 128          # 64 block-rows
SPB = NBLK // NCORES      # 8 strips (block-rows) per core
NT = 33                   # tiles per strip (c = 0..32)
SW = NT * 128             # 4224 strip width (stream columns per strip)
TOTAL = SPB * SW          # 33792 stream columns per core
LCOLS = 128 * (SPB - 1) + SW   # 5120 local rhs columns per core
SCALE = 16.0              # fp8 pre-scale on z
ACT_SCALE = (1.0 / TEMP) / (SCALE * SCALE)   # exp(s_hat * ACT_SCALE)
CS_LO = 128               # colsum window (local cols): [128j+128, 128j+4096)
CS_HI = 128 * (SPB - 1) + 4096   # 4992
CSW = CS_HI - CS_LO       # 4864
CPH = [(4224, 4992), (128, 2176), (2176, 4224)]   # colsum column phases
ROUNDS = [(0, 1024), (1024, 2048), (2048, 3072), (3072, 4224), (4224, LCOLS)]

_CACHE = {}


def _build():
    import concourse.bacc as bacc
    import concourse.mybir as mybir
    import concourse.tile as tile

    f32 = mybir.dt.float32
    bf16 = mybir.dt.bfloat16
    fp8 = mybir.dt.float8e4
    AF = mybir.ActivationFunctionType
    ALU = mybir.AluOpType
    X = mybir.AxisListType.X
    DR = mybir.MatmulPerfMode.DoubleRow

    nc = bacc.Bacc("TRN2", target_bir_lowering=False, debug=False,
                   num_devices=NCORES)

    xq_d = nc.dram_tensor("xq", [128, 4 * LCOLS], fp8, kind="ExternalInput").ap()
    rowout_d = nc.dram_tensor("rowout", [128, SPB], f32,
                              kind="ExternalOutput").ap()
    colout_d = nc.dram_tensor("colout", [1, CSW], f32,
                              kind="ExternalOutput").ap()

    with tile.TileContext(nc) as tc:
        with (
            tc.tile_pool(name="persist", bufs=1) as P,
            tc.tile_pool(name="scratch", bufs=2) as S,
            tc.tile_pool(name="psum", bufs=2, space="PSUM") as PS,
        ):
            ones = P.tile([128, 128], bf16, name="ones")
            xq = P.tile([128, 4 * LCOLS], fp8, name="xq")
            exps = P.tile([128, TOTAL], bf16, name="exps")
            rowacc = P.tile([128, SPB], f32, name="rowacc")
            colsb = P.tile([128, 512 * len(CPH)], f32, name="colsb")
            fa7 = P.tile([128, 1056], bf16, name="fa7")

            nc.vector.memset(ones[:], 1.0)
            # HBM loads, first-needed first, round-robined over three DMA
            # queues (the scalar DGE is idle until the first activation).
            qs = [nc.sync, nc.gpsimd, nc.scalar]
            qi = 0
            for lo, hi in ROUNDS:
                for s in range(4):
                    sl = slice(LCOLS * s + lo, LCOLS * s + hi)
                    qs[qi % 3].dma_start(xq[:, sl], xq_d[:, sl])
                    qi += 1

            # warm the PE HAM clock gate while the first loads are in flight
            wps = PS.tile([128, 128], f32, tag="mm", name="wps")
            for _ in range(24):
                nc.tensor.matmul(wps[:], ones[:], ones[:], start=True, stop=True)

            xq3 = xq[:].rearrange("p (s c) -> p s c", s=4)

            def fold(j, dst_ap, q):
                """DVE pairwise bf16 tree folds for strip j row sums.
                q = (q0+q2, q1+q3) quarters order so half the work can be
                emitted a chunk early for the final strip."""
                a = exps[:, SW * j:SW * (j + 1)]
                with nc.allow_low_precision(
                        reason="pairwise bf16 folds; final add is f32"):
                    f2 = S.tile([128, 1056], bf16, tag="f2", name="f2")
                    if q is None:
                        f1 = S.tile([128, 2112], bf16, tag="f1", name="f1")
                        nc.vector.tensor_tensor(
                            f1[:], a[:, 0:2112], a[:, 2112:4224], ALU.add)
                        nc.vector.tensor_tensor(
                            f2[:], f1[:, 0:1056], f1[:, 1056:2112], ALU.add)
                    else:
                        fb = S.tile([128, 1056], bf16, tag="f1", name="fb")
                        nc.vector.tensor_tensor(
                            fb[:], a[:, 1056:2112], a[:, 3168:4224], ALU.add)
                        nc.vector.tensor_tensor(f2[:], q[:], fb[:], ALU.add)
                    f3 = S.tile([128, 528], bf16, tag="f3", name="f3")
                    nc.vector.tensor_tensor(
                        f3[:], f2[:, 0:528], f2[:, 528:1056], ALU.add)
                nc.vector.tensor_reduce(dst_ap, f3[:], X, ALU.add)

            bounds = ([0, 1024, 2048, 3072] +
                      list(range(4096, 32768 + 1, 2048)) + [33280, TOTAL])
            cuts = sorted(set(
                [512 * m for m in range(TOTAL // 512 + 1)] +
                [SW * j for j in range(SPB + 1)]))
            pieces = list(zip(cuts, cuts[1:]))
            reduced = set()
            fa_done = False
            for c0, c1 in zip(bounds, bounds[1:]):
                cp = [pc for pc in pieces if pc[0] >= c0 and pc[1] <= c1]
                ps = PS.tile([128, c1 - c0], f32, tag="mm", name="ps")
                emit = []
                for j in sorted(set(a // SW for a, b in cp)):
                    for kk in range(2):
                        for (a, b) in cp:
                            if a // SW == j:
                                emit.append((j, kk, a, b))
                regions = {}
                for idx, (j, kk, a, b) in enumerate(emit):
                    regions.setdefault(a // 512, []).append(idx)
                starts = {v[0] for v in regions.values()}
                stops = {v[-1] for v in regions.values()}
                for idx, (j, kk, a, b) in enumerate(emit):
                    lc = a - 4096 * j
                    nc.tensor.matmul(
                        ps[:, a - c0:b - c0],
                        xq3[:, 2 * kk:2 * kk + 2, 128 * j:128 * (j + 1)],
                        xq3[:, 2 * kk:2 * kk + 2, lc:lc + (b - a)],
                        start=(idx in starts), stop=(idx in stops),
                        perf_mode=DR)
                nc.scalar.activation(exps[:, c0:c1], ps[:, 0:c1 - c0],
                                     AF.Exp, scale=ACT_SCALE)
                for j in range(SPB - 1):
                    if j not in reduced and SW * (j + 1) <= c1:
                        fold(j, rowacc[:, j:j + 1], None)
                        reduced.add(j)
                if not fa_done and SW * (SPB - 1) + 3168 <= c1:
                    # front-load half of the last strip's fold: q0 + q2
                    b7 = SW * (SPB - 1)
                    with nc.allow_low_precision(reason="bf16 fold half"):
                        nc.vector.tensor_tensor(
                            fa7[:], exps[:, b7:b7 + 1056],
                            exps[:, b7 + 2112:b7 + 3168], ALU.add)
                    fa_done = True

            # column-sum tail: 4 concurrent M=32 all-ones matmuls on distinct
            # PE column groups, strips accumulated in shared PSUM per phase.
            # The last strip's remaining fold is slotted between phase 1 and
            # phase 2 so neither the copies nor the row sums gate the end.
            for p, (pa, pb) in enumerate(CPH):
                if p == 2:
                    fold(SPB - 1, rowacc[:, SPB - 1:SPB], fa7)
                    nc.gpsimd.dma_start(rowout_d[:], rowacc[:])
                cps = PS.tile([128, 512], f32, tag="mm", name="cps")
                emit = []
                for j in range(SPB):
                    for g in range(4):
                        ga, gb = pa + 512 * g, min(pb, pa + 512 * (g + 1))
                        wa = max(ga, 128 * j + 128)
                        wb = min(gb, 128 * j + 4096)
                        if wa < wb:
                            emit.append((g, j, wa, wb))
                groups = {}
                for idx, (g, j, a, b) in enumerate(emit):
                    groups.setdefault(g, []).append(idx)
                starts = {v[0] for v in groups.values()}
                stops = {v[-1] for v in groups.values()}
                for idx, (g, j, a, b) in enumerate(emit):
                    ga = pa + 512 * g
                    nc.tensor.matmul(
                        cps[32 * g:32 * (g + 1), a - ga:b - ga],
                        ones[:, 0:32], exps[:, 4096 * j + a:4096 * j + b],
                        start=(idx in starts), stop=(idx in stops),
                        tile_position=(0, 32 * g))
                csl = 512 * p
                # scalar engine is idle once the last exp chunk is done;
                # draining psum there keeps DVE free for the row-sum folds
                nc.scalar.copy(colsb[:, csl:csl + 512], cps[:])
                w = pb - pa
                eng = nc.sync if p % 2 == 0 else nc.gpsimd
                if w == 2048:
                    src = colsb[:].rearrange("(a b) f -> a b f", b=32)[
                        :, 0:1, csl:csl + 512]
                    eng.dma_start(colout_d[0:1, pa - CS_LO:pb - CS_LO], src)
                else:
                    for g in range(4):
                        ga, gb = pa + 512 * g, min(pb, pa + 512 * (g + 1))
                        if ga >= gb:
                            continue
                        eng.dma_start(
                            colout_d[0:1, ga - CS_LO:gb - CS_LO],
                            colsb[32 * g:32 * g + 1, csl:csl + (gb - ga)])

    nc.compile()
    return nc


def _get_nc():
    if "nc" not in _CACHE:
        _CACHE["nc"] = _build()
    return _CACHE["nc"]


def _prep(emb_i, emb_j):
    fp8 = ml_dtypes.float8_e4m3
    E = np.concatenate([np.asarray(emb_i, dtype=np.float32),
                        np.asarray(emb_j, dtype=np.float32)], axis=0)
    nrm = np.sqrt((E * E).sum(axis=1, keepdims=True))
    Z = E / np.maximum(nrm, 1e-12)                       # [8192, 512] f32
    pos = (Z[:BATCH] * Z[BATCH:]).sum(axis=1)
    posf = np.concatenate([pos, pos]) / TEMP             # [8192]
    Zq = (Z * SCALE).astype(fp8)                         # [8192, 512] fp8
    Zqf = Zq.astype(np.float32) / SCALE
    diag = np.exp((Zqf * Zqf).sum(axis=1) / TEMP)        # [8192]
    ZqT = np.ascontiguousarray(Zq.T)                     # [512, 8192]
    maps = []
    for k in range(NCORES):
        cols = (1024 * k + np.arange(LCOLS)) % B2
        Xc = ZqT[:, cols]                                # [512, 5120]
        xq = np.ascontiguousarray(
            Xc.reshape(4, 128, LCOLS).transpose(1, 0, 2).reshape(128, -1))
        maps.append({"xq": xq})
    return maps, posf, diag


def _run(emb_i, emb_j, trace=False):
    from concourse.bass_utils import run_bass_kernel_spmd
    nc = _get_nc()
    maps, posf, diag = _prep(emb_i, emb_j)
    res = run_bass_kernel_spmd(nc, maps, list(range(NCORES)), trace=trace)
    den = np.zeros(B2, dtype=np.float64)
    for k in range(NCORES):
        rowout = np.asarray(res.results[k]["rowout"], dtype=np.float64)
        colout = np.asarray(res.results[k]["colout"], dtype=np.float64)[0]
        rows = 1024 * k + np.arange(1024)
        den[rows] += rowout.T.reshape(-1)                # [p, j] -> row 128j+p
        g = (1024 * k + CS_LO + np.arange(CSW)) % B2
        den[g] += colout
    den = den - diag
    loss = np.float32(np.mean(np.log(den) - posf))
    return loss, res


def kernel(emb_i, emb_j):
    return _run(emb_i, emb_j, trace=False)[0]
